# revision 13
# baseline (speedup 1.0000x reference)
"""Trainium2 Bass kernel for nn_AttentionSup (dense transformer attention block).

Computation (see reference):
  qkv = x @ W_qkv; per-head attention softmax(q k^T / sqrt(d)) v;
  domain-gate (tiny MLP + softmax over heads) multiplies the attention
  output per (batch, head, dim); out = gated @ W_out + b_out.

Sharding over 8 NeuronCores: (batch b in 0..3) x (head-group g in 0..1),
4 heads per core — data-parallel over batch, tensor-parallel over heads.
Each core computes a partial output [2048, 512] for its batch from its 4
heads; the host sums the two head-group partials per batch and adds b_out
(the "all-reduce after to_out", done on host since partials per batch live
on exactly 2 cores).

The tiny domain-gate MLP ([4x4] @ [4x32] @ [32x512] per batch) is computed
on the host and folded into the V projection weights (it scales O columns,
i.e. W_v columns). Softmax uses the unnormalized-exp + ones-column trick:
V_ext = [V | 1], so PV matmul also produces row sums; normalization is a
reciprocal broadcast multiply. exp skips max-subtraction (scores ~N(0,1),
max ~5 — no overflow risk in fp32).

All matmuls run in fp32r (1 cycle/row at free-dim>=256; fp32 would be 4x
slower). fp32 PSUM accumulation throughout.
"""

import sys

sys.path.insert(0, "/opt/trn_rl_repo")

import numpy as np
from contextlib import ExitStack

import concourse.bass as bass
import concourse.tile as tile
from concourse import bacc, mybir
from concourse.bass_utils import run_bass_kernel_spmd


def _install_ntff_hook():
    """Provide antenv.axon_hooks (absent from the image) so
    run_bass_kernel_spmd(trace=True) can capture NTFF profiles under axon."""
    import types

    if "antenv.axon_hooks" in sys.modules:
        return
    mod = types.ModuleType("antenv.axon_hooks")
    mod._HOOK = None
    mod.set_axon_ntff_profile_hook = lambda h: setattr(mod, "_HOOK", h)
    mod.get_axon_ntff_profile_hook = lambda: mod._HOOK
    try:
        from trn_agent_boot.trn_boot import _ntff_profile_via_ctypes

        mod._HOOK = _ntff_profile_via_ctypes("/opt/axon/libaxon_pjrt.so")
    except Exception:
        pass
    sys.modules["antenv.axon_hooks"] = mod
    try:
        import antenv

        antenv.axon_hooks = mod
    except Exception:
        pass


_install_ntff_hook()

f32 = mybir.dt.float32
f32r = mybir.dt.float32r
Exp = mybir.ActivationFunctionType.Exp

# Problem shapes (hardcoded per contract)
B, N, D = 4, 2048, 512
HEADS, DH = 8, 64
INNER = HEADS * DH  # 512
SCALE = DH**-0.5
NCORES = 8
HG = 2  # head groups (tensor-parallel degree)
HPC = HEADS // HG  # 4 heads per core
F = HPC * DH  # 256 inner dims per core
NT = N // 128  # 16 n(token)-tiles
DT = D // 128  # 4 d-tiles
QC = 4  # q chunks of 512
KTP = NT // 2  # 8 kt-pairs

_NC_CACHE = {}


def _build():
    """Build + compile the per-core Bass program (same program on all cores)."""
    nc = bacc.Bacc("TRN2", target_bir_lowering=False, debug=False, num_devices=NCORES)

    xT_d = nc.dram_tensor("xT", [D, N], f32, kind="ExternalInput")
    wq_d = nc.dram_tensor("wq", [D, F], f32, kind="ExternalInput")
    wk_d = nc.dram_tensor("wk", [D, F], f32, kind="ExternalInput")
    wv_d = nc.dram_tensor("wv", [D, F], f32, kind="ExternalInput")  # gate-scaled
    wo_d = nc.dram_tensor("wo", [F, D], f32, kind="ExternalInput")
    ones_d = nc.dram_tensor("ones", [128, 64], f32, kind="ExternalInput")
    part_d = nc.dram_tensor("part", [N, D], f32, kind="ExternalOutput")

    with tile.TileContext(nc) as tc:
        with ExitStack() as ctx:
            persist = ctx.enter_context(tc.tile_pool(name="persist", bufs=1))

            # qT/kT: [f, n] layout, one tile per head-pair (f-tile).
            qt_sb = [
                persist.tile([128, N], f32r, tag=f"qt{i}", name=f"qt{i}")
                for i in range(2)
            ]
            kt_sb = [
                persist.tile([128, N], f32r, tag=f"kt{i}", name=f"kt{i}")
                for i in range(2)
            ]
            # V_ext natural layout: [ktok, nt, head, 64+1]
            v_sb = persist.tile([128, NT, HPC, 65], f32r, tag="v", name="v")
            # gated+normalized attention output O^T: [f, n], per head-pair
            og_sb = [
                persist.tile([128, N], f32r, tag=f"og{i}", name=f"og{i}")
                for i in range(2)
            ]
            wo_sb = persist.tile([128, 2, D], f32r, tag="wo", name="wo")
            ones_sb = persist.tile([1, 64], f32r, tag="ones1", name="ones1")

            nc.sync.dma_start(ones_sb[:], ones_d[0:1, :].bitcast(f32r))
            # Warm the ACT exp table set (~2.7us load) during the input DMAs
            # so the first real exp in phase 2 doesn't stall on it.
            warm_sb = persist.tile([1, 64], f32, tag="warm", name="warm")
            nc.scalar.activation(warm_sb[:], ones_sb[:].bitcast(f32), Exp, scale=0.0)
            nc.sync.dma_start(
                wo_sb[:], wo_d[:].rearrange("(ft p) m -> p ft m", p=128).bitcast(f32r)
            )
            # ones column of V_ext, one strided DMA
            nc.sync.dma_start(
                v_sb[:].rearrange("p a b c -> p (a b) c")[:, :, 64],
                ones_d[:, :].bitcast(f32r),
            )

            # ---------------- Phase 1: QKV projections ----------------
            with (
                tc.tile_pool(name="ph1", bufs=1) as ph1,
                tc.tile_pool(name="ps1", bufs=8, space="PSUM") as ps1,
            ):
                # chunk the xT DMA per d-tile AND use one tile per chunk so
                # dependency tracking lets the first matmuls start after ~1MB
                # instead of waiting for the whole 4MB
                xt_sb = [
                    ph1.tile([128, N], f32r, tag=f"xt{dt}", name=f"xt{dt}")
                    for dt in range(DT)
                ]
                w_sb = {}
                for wname, w_d in (("wq", wq_d), ("wk", wk_d), ("wv", wv_d)):
                    w_sb[wname] = [
                        ph1.tile([128, F], f32r, tag=f"{wname}{dt}", name=f"{wname}{dt}")
                        for dt in range(DT)
                    ]
                xt_r = xT_d[:].rearrange("(dt p) n -> p dt n", p=128).bitcast(f32r)
                for dt in range(DT):
                    nc.sync.dma_start(xt_sb[dt][:], xt_r[:, dt])
                    for wname, w_d in (("wq", wq_d), ("wk", wk_d), ("wv", wv_d)):
                        nc.sync.dma_start(
                            w_sb[wname][dt][:],
                            w_d[:]
                            .rearrange("(dt p) f -> p dt f", p=128)
                            .bitcast(f32r)[:, dt],
                        )

                # qT/kT: [f, n] = W^T x^T ; lhsT = W[dtile, ftile], rhs =
                # xT[dtile, nchunk]. dt-outer over 8 live psum accumulators:
                # PE starts on the first xT chunk.
                for wname, dst in (("wq", qt_sb), ("wk", kt_sb)):
                    tiles = [
                        ps1.tile([128, 512], f32, tag="mm", name=f"mm_ps{i}")
                        for i in range(8)
                    ]
                    for dt in range(DT):
                        for ft in range(2):
                            for qc in range(QC):
                                nc.tensor.matmul(
                                    tiles[ft * QC + qc][:],
                                    w_sb[wname][dt][:, ft * 128 : (ft + 1) * 128],
                                    xt_sb[dt][:, qc * 512 : (qc + 1) * 512],
                                    start=(dt == 0),
                                    stop=(dt == DT - 1),
                                )
                    for ft in range(2):
                        for qc in range(QC):
                            nc.vector.tensor_copy(
                                dst[ft][:, qc * 512 : (qc + 1) * 512],
                                tiles[ft * QC + qc][:],
                            )

                # V natural: [n, f] = x W_v ; lhsT = xT[dtile, ntile], rhs = W_v[dtile, :]
                for nt in range(NT):
                    ps = ps1.tile([128, 512], f32, tag="mm", name="mmv_ps")
                    for dt in range(DT):
                        nc.tensor.matmul(
                            ps[:, 0:F],
                            xt_sb[dt][:, nt * 128 : (nt + 1) * 128],
                            w_sb["wv"][dt][:],
                            start=(dt == 0),
                            stop=(dt == DT - 1),
                        )
                    nc.vector.tensor_copy(
                        v_sb[:, nt, :, 0:64],
                        ps[:, 0:F].rearrange("p (h e) -> p h e", e=64),
                    )

            # ---------------- Phase 2: attention ----------------
            # psS bufs=3 (6 banks) lets PE run ST matmuls ~2 kt-pairs ahead
            # of the exp on ACT, keeping the tensor engine dense enough to
            # hold the HAM clock at 2.4 GHz. psO bufs=2 (2 banks) pipelines
            # the per-(head, qchunk) accumulator across iterations.
            with (
                tc.tile_pool(name="ptp", bufs=4) as ptp,
                tc.tile_pool(name="normp", bufs=2) as normp,
                tc.tile_pool(name="psS", bufs=3, space="PSUM") as psS,
                tc.tile_pool(name="psO", bufs=2, space="PSUM") as psO,
            ):
                def pv_pair(o_ps, pt, hp, h01, ktp):
                    for j in range(2):
                        kt = 2 * ktp + j
                        nc.tensor.matmul(
                            o_ps[:],
                            v_sb[:, kt, hp * 2 + h01, :],
                            pt[:, j * 512 : (j + 1) * 512],
                            start=(kt == 0),
                            stop=(kt == NT - 1),
                        )

                for hp in range(2):
                    for qc in range(QC):
                        for h01 in range(2):
                            off = h01 * 64
                            o_ps = psO.tile([65, 512], f32, tag="O", name="o_ps")
                            # software pipeline: PV consumes the PREVIOUS
                            # kt-pair's exp output so the tensor engine never
                            # waits on the activation engine mid-stream; one
                            # keep-warm filler matmul per iteration balances
                            # PE pace to the exp pace so the HAM clock gate
                            # stays at 2.4 GHz.
                            prev = None
                            for ktp in range(KTP):
                                s_ps = psS.tile([128, 1024], f32, tag="S", name="s_ps")
                                # keep-warm filler matmul into the S tile
                                # (overwritten by the real ST matmuls below):
                                # keeps the PE activity window dense so the
                                # HAM clock gate stays at 2.4 GHz during the
                                # exp-paced attention stream.
                                nc.tensor.matmul(
                                    s_ps[:, 0:512],
                                    wo_sb[:, 0, 0:128],
                                    wo_sb[:, 0, :],
                                    start=True,
                                    stop=True,
                                )
                                for j in range(2):
                                    kt = 2 * ktp + j
                                    nc.tensor.matmul(
                                        s_ps[:, j * 512 : (j + 1) * 512],
                                        kt_sb[hp][
                                            off : off + 64, kt * 128 : (kt + 1) * 128
                                        ],
                                        qt_sb[hp][
                                            off : off + 64, qc * 512 : (qc + 1) * 512
                                        ],
                                        start=True,
                                        stop=True,
                                    )
                                pt = ptp.tile([128, 1024], f32r, tag="PT", name="pt")
                                nc.scalar.activation(pt[:], s_ps[:], Exp, scale=SCALE)
                                if prev is not None:
                                    pv_pair(o_ps, prev, hp, h01, ktp - 1)
                                prev = pt
                            pv_pair(o_ps, prev, hp, h01, KTP - 1)
                            # normalize: og = O[0:64] * (1 / sums), sums bcast
                            # via K=1 matmul with a ones lhsT
                            srow = normp.tile([1, 512], f32r, tag="srow", name="srow")
                            nc.vector.tensor_copy(srow[:], o_ps[64:65, :])
                            rs_ps = psS.tile([64, 512], f32, tag="S", name="rs_ps")
                            nc.tensor.matmul(
                                rs_ps[:], ones_sb[:], srow[:], start=True, stop=True
                            )
                            rinv = normp.tile([64, 512], f32, tag="rinv", name="rinv")
                            nc.vector.reciprocal_approx_fast(rinv[:], rs_ps[:])
                            nc.vector.tensor_tensor(
                                og_sb[hp][off : off + 64, qc * 512 : (qc + 1) * 512],
                                o_ps[0:64, :],
                                rinv[:],
                                mybir.AluOpType.mult,
                            )

            # ---------------- Phase 3: output projection ----------------
            with (
                tc.tile_pool(name="ps3", bufs=4, space="PSUM") as ps3,
                tc.tile_pool(name="out3", bufs=4) as out3,
            ):
                for nt in range(NT):
                    ps = ps3.tile([128, 512], f32, tag="fin", name="fin_ps")
                    for hp in range(2):
                        nc.tensor.matmul(
                            ps[:],
                            og_sb[hp][:, nt * 128 : (nt + 1) * 128],
                            wo_sb[:, hp, :],
                            start=(hp == 0),
                            stop=(hp == 1),
                        )
                    ob = out3.tile([128, 512], f32, tag="ob", name="ob")
                    nc.vector.tensor_copy(ob[:], ps[:])
                    nc.sync.dma_start(part_d[nt * 128 : (nt + 1) * 128, :], ob[:])

    nc.compile()
    return nc


def _get_nc():
    if "nc" not in _NC_CACHE:
        _NC_CACHE["nc"] = _build()
    return _NC_CACHE["nc"]


def _prepare_in_maps(x, domain_label, W_qkv, W_d1, b_d1, W_d2, b_d2, W_out, b_out):
    x = np.asarray(x, np.float32)
    domain_label = np.asarray(domain_label, np.float32)
    W_qkv = np.asarray(W_qkv, np.float32)
    W_d1 = np.asarray(W_d1, np.float32)
    b_d1 = np.asarray(b_d1, np.float32)
    W_d2 = np.asarray(W_d2, np.float32)
    b_d2 = np.asarray(b_d2, np.float32)
    W_out = np.asarray(W_out, np.float32)

    # host: domain gate MLP + softmax over heads (tiny)
    d1 = np.maximum(domain_label @ W_d1 + b_d1, 0.0)
    d = d1 @ W_d2 + b_d2  # [B, INNER]
    d = d.reshape(B, HEADS, DH)
    e = np.exp(d - d.max(axis=1, keepdims=True))
    gate = (e / e.sum(axis=1, keepdims=True)).reshape(B, INNER).astype(np.float32)

    ones = np.ones((128, 64), np.float32)
    in_maps = []
    for c in range(NCORES):
        b, g = c // HG, c % HG
        sl = slice(g * F, (g + 1) * F)
        in_maps.append(
            {
                "xT": np.ascontiguousarray(x[b].T),
                "wq": np.ascontiguousarray(W_qkv[:, sl]),
                "wk": np.ascontiguousarray(W_qkv[:, INNER:][:, sl]),
                "wv": np.ascontiguousarray(
                    W_qkv[:, 2 * INNER :][:, sl] * gate[b, sl][None, :]
                ),
                "wo": np.ascontiguousarray(W_out[sl, :]),
                "ones": ones,
            }
        )
    return in_maps


def _run(in_maps, trace=False, tmpdir=None):
    nc = _get_nc()
    return run_bass_kernel_spmd(
        nc, in_maps, list(range(NCORES)), trace=trace, tmpdir=tmpdir
    )


def _assemble(results, b_out):
    b_out = np.asarray(b_out, np.float32)
    out = np.empty((B, N, D), np.float32)
    for b in range(B):
        out[b] = results[HG * b]["part"] + results[HG * b + 1]["part"] + b_out
    return out


def kernel(x, domain_label, W_qkv, W_d1, b_d1, W_d2, b_d2, W_out, b_out):
    in_maps = _prepare_in_maps(
        x, domain_label, W_qkv, W_d1, b_d1, W_d2, b_d2, W_out, b_out
    )
    res = _run(in_maps, trace=False)
    return _assemble(res.results, b_out)


# revision 14
# speedup vs baseline: 1.2181x; 1.2181x over previous
"""Trainium2 Bass kernel for nn_AttentionSup (dense transformer attention block).

Computation (see reference):
  qkv = x @ W_qkv; per-head attention softmax(q k^T / sqrt(d)) v;
  domain-gate (tiny MLP + softmax over heads) multiplies the attention
  output per (batch, head, dim); out = gated @ W_out + b_out.

Sharding over 8 NeuronCores: (batch b in 0..3) x (head-group g in 0..1),
4 heads per core — data-parallel over batch, tensor-parallel over heads.
Each core computes a partial output [2048, 512] for its batch from its 4
heads; the host sums the two head-group partials per batch and adds b_out
(the "all-reduce after to_out", done on host since partials per batch live
on exactly 2 cores).

The tiny domain-gate MLP ([4x4] @ [4x32] @ [32x512] per batch) is computed
on the host and folded into the V projection weights (it scales O columns,
i.e. W_v columns). Softmax uses the unnormalized-exp + ones-column trick:
V_ext = [V | 1], so PV matmul also produces row sums; normalization is a
reciprocal broadcast multiply. exp skips max-subtraction (scores ~N(0,1),
max ~5 — no overflow risk in fp32).

All matmuls run in fp32r (1 cycle/row at free-dim>=256; fp32 would be 4x
slower). fp32 PSUM accumulation throughout.
"""

import sys

sys.path.insert(0, "/opt/trn_rl_repo")

import numpy as np
from contextlib import ExitStack

import concourse.bass as bass
import concourse.tile as tile
from concourse import bacc, mybir
from concourse.bass_utils import run_bass_kernel_spmd


def _install_ntff_hook():
    """Provide antenv.axon_hooks (absent from the image) so
    run_bass_kernel_spmd(trace=True) can capture NTFF profiles under axon."""
    import types

    if "antenv.axon_hooks" in sys.modules:
        return
    mod = types.ModuleType("antenv.axon_hooks")
    mod._HOOK = None
    mod.set_axon_ntff_profile_hook = lambda h: setattr(mod, "_HOOK", h)
    mod.get_axon_ntff_profile_hook = lambda: mod._HOOK
    try:
        from trn_agent_boot.trn_boot import _ntff_profile_via_ctypes

        mod._HOOK = _ntff_profile_via_ctypes("/opt/axon/libaxon_pjrt.so")
    except Exception:
        pass
    sys.modules["antenv.axon_hooks"] = mod
    try:
        import antenv

        antenv.axon_hooks = mod
    except Exception:
        pass


_install_ntff_hook()

f32 = mybir.dt.float32
f32r = mybir.dt.float32r
Exp = mybir.ActivationFunctionType.Exp

# Problem shapes (hardcoded per contract)
B, N, D = 4, 2048, 512
HEADS, DH = 8, 64
INNER = HEADS * DH  # 512
SCALE = DH**-0.5
NCORES = 8
HG = 2  # head groups (tensor-parallel degree)
HPC = HEADS // HG  # 4 heads per core
F = HPC * DH  # 256 inner dims per core
NT = N // 128  # 16 n(token)-tiles
DT = D // 128  # 4 d-tiles
QC = 4  # q chunks of 512
KTP = NT // 2  # 8 kt-pairs

_NC_CACHE = {}


def _build():
    """Build + compile the per-core Bass program (same program on all cores)."""
    nc = bacc.Bacc("TRN2", target_bir_lowering=False, debug=False, num_devices=NCORES)

    xT_d = nc.dram_tensor("xT", [D, N], f32, kind="ExternalInput")
    wq_d = nc.dram_tensor("wq", [D, F], f32, kind="ExternalInput")
    wk_d = nc.dram_tensor("wk", [D, F], f32, kind="ExternalInput")
    wv_d = nc.dram_tensor("wv", [D, F], f32, kind="ExternalInput")  # gate-scaled
    wo_d = nc.dram_tensor("wo", [F, D], f32, kind="ExternalInput")
    ones_d = nc.dram_tensor("ones", [128, 64], f32, kind="ExternalInput")
    part_d = nc.dram_tensor("part", [N, D], f32, kind="ExternalOutput")

    with tile.TileContext(nc) as tc:
        with ExitStack() as ctx:
            persist = ctx.enter_context(tc.tile_pool(name="persist", bufs=1))

            # qT/kT: [f, n] layout, one tile per head-pair (f-tile).
            qt_sb = [
                persist.tile([128, N], f32r, tag=f"qt{i}", name=f"qt{i}")
                for i in range(2)
            ]
            kt_sb = [
                persist.tile([128, N], f32r, tag=f"kt{i}", name=f"kt{i}")
                for i in range(2)
            ]
            # V_ext natural layout: [ktok, nt, head, 64+1]
            v_sb = persist.tile([128, NT, HPC, 65], f32r, tag="v", name="v")
            # gated+normalized attention output O^T: [f, n], per head-pair
            og_sb = [
                persist.tile([128, N], f32r, tag=f"og{i}", name=f"og{i}")
                for i in range(2)
            ]
            wo_sb = persist.tile([128, 2, D], f32r, tag="wo", name="wo")
            ones_sb = persist.tile([1, 64], f32r, tag="ones1", name="ones1")

            nc.sync.dma_start(ones_sb[:], ones_d[0:1, :].bitcast(f32r))
            # Warm the ACT exp table set (~2.7us load) during the input DMAs
            # so the first real exp in phase 2 doesn't stall on it.
            warm_sb = persist.tile([1, 64], f32, tag="warm", name="warm")
            nc.scalar.activation(warm_sb[:], ones_sb[:].bitcast(f32), Exp, scale=0.0)
            nc.sync.dma_start(
                wo_sb[:], wo_d[:].rearrange("(ft p) m -> p ft m", p=128).bitcast(f32r)
            )
            # ones column of V_ext, one strided DMA
            nc.sync.dma_start(
                v_sb[:].rearrange("p a b c -> p (a b) c")[:, :, 64],
                ones_d[:, :].bitcast(f32r),
            )

            # ---------------- Phase 1: QKV projections ----------------
            with (
                tc.tile_pool(name="ph1", bufs=1) as ph1,
                tc.tile_pool(name="ps1", bufs=8, space="PSUM") as ps1,
            ):
                # chunk the xT DMA per d-tile AND use one tile per chunk so
                # dependency tracking lets the first matmuls start after ~1MB
                # instead of waiting for the whole 4MB
                xt_sb = [
                    ph1.tile([128, N], f32r, tag=f"xt{dt}", name=f"xt{dt}")
                    for dt in range(DT)
                ]
                w_sb = {}
                for wname, w_d in (("wq", wq_d), ("wk", wk_d), ("wv", wv_d)):
                    w_sb[wname] = [
                        ph1.tile([128, F], f32r, tag=f"{wname}{dt}", name=f"{wname}{dt}")
                        for dt in range(DT)
                    ]
                xt_r = xT_d[:].rearrange("(dt p) n -> p dt n", p=128).bitcast(f32r)
                for dt in range(DT):
                    for wname, w_d in (("wq", wq_d), ("wk", wk_d), ("wv", wv_d)):
                        nc.sync.dma_start(
                            w_sb[wname][dt][:],
                            w_d[:]
                            .rearrange("(dt p) f -> p dt f", p=128)
                            .bitcast(f32r)[:, dt],
                        )
                for dt in range(DT):
                    nc.sync.dma_start(xt_sb[dt][:], xt_r[:, dt])

                # qT/kT: [f, n] = W^T x^T ; lhsT = W[dtile, ftile], rhs =
                # xT[dtile, nchunk]. dt-outer over 8 live psum accumulators:
                # PE starts on the first xT chunk.
                for wname, dst in (("wq", qt_sb), ("wk", kt_sb)):
                    tiles = [
                        ps1.tile([128, 512], f32, tag="mm", name=f"mm_ps{i}")
                        for i in range(8)
                    ]
                    for dt in range(DT):
                        for ft in range(2):
                            for qc in range(QC):
                                nc.tensor.matmul(
                                    tiles[ft * QC + qc][:],
                                    w_sb[wname][dt][:, ft * 128 : (ft + 1) * 128],
                                    xt_sb[dt][:, qc * 512 : (qc + 1) * 512],
                                    start=(dt == 0),
                                    stop=(dt == DT - 1),
                                )
                    for ft in range(2):
                        for qc in range(QC):
                            nc.vector.tensor_copy(
                                dst[ft][:, qc * 512 : (qc + 1) * 512],
                                tiles[ft * QC + qc][:],
                            )

                # V natural: [n, f] = x W_v ; lhsT = xT[dtile, ntile], rhs = W_v[dtile, :]
                for nt in range(NT):
                    ps = ps1.tile([128, 512], f32, tag="mm", name="mmv_ps")
                    for dt in range(DT):
                        nc.tensor.matmul(
                            ps[:, 0:F],
                            xt_sb[dt][:, nt * 128 : (nt + 1) * 128],
                            w_sb["wv"][dt][:],
                            start=(dt == 0),
                            stop=(dt == DT - 1),
                        )
                    nc.vector.tensor_copy(
                        v_sb[:, nt, :, 0:64],
                        ps[:, 0:F].rearrange("p (h e) -> p h e", e=64),
                    )

            # ---------------- Phase 2: attention ----------------
            # psS bufs=3 (6 banks) lets PE run ST matmuls ~2 kt-pairs ahead
            # of the exp on ACT, keeping the tensor engine dense enough to
            # hold the HAM clock at 2.4 GHz. psO bufs=2 (2 banks) pipelines
            # the per-(head, qchunk) accumulator across iterations.
            with (
                tc.tile_pool(name="ptp", bufs=4) as ptp,
                tc.tile_pool(name="normp", bufs=2) as normp,
                tc.tile_pool(name="psS", bufs=2, space="PSUM") as psS,
                tc.tile_pool(name="psO", bufs=2, space="PSUM") as psO,
                tc.tile_pool(name="psD", bufs=1, space="PSUM") as psD,
            ):
                def pv_pair(o_ps, pt, hp, h01, ktp):
                    for j in range(2):
                        kt = 2 * ktp + j
                        nc.tensor.matmul(
                            o_ps[:],
                            v_sb[:, kt, hp * 2 + h01, :],
                            pt[:, j * 512 : (j + 1) * 512],
                            start=(kt == 0),
                            stop=(kt == NT - 1),
                        )

                for hp in range(2):
                    for qc in range(QC):
                        for h01 in range(2):
                            off = h01 * 64
                            o_ps = psO.tile([65, 512], f32, tag="O", name="o_ps")
                            # software pipeline: PV consumes the PREVIOUS
                            # kt-pair's exp output so the tensor engine never
                            # waits on the activation engine mid-stream; one
                            # keep-warm filler matmul per iteration balances
                            # PE pace to the exp pace so the HAM clock gate
                            # stays at 2.4 GHz.
                            prev = None
                            for ktp in range(KTP):
                                s_ps = psS.tile([128, 1024], f32, tag="S", name="s_ps")
                                for j in range(2):
                                    kt = 2 * ktp + j
                                    nc.tensor.matmul(
                                        s_ps[:, j * 512 : (j + 1) * 512],
                                        kt_sb[hp][
                                            off : off + 64, kt * 128 : (kt + 1) * 128
                                        ],
                                        qt_sb[hp][
                                            off : off + 64, qc * 512 : (qc + 1) * 512
                                        ],
                                        start=True,
                                        stop=True,
                                    )
                                # two keep-warm filler matmuls per iteration:
                                # attention is exp(ACT)-paced; without dense
                                # PE work the HAM clock gate falls to 1.2 GHz
                                # (2x matmul slowdown). Two junk matmuls keep
                                # the PE strictly busier than ACT so the
                                # activity window stays warm.
                                d_ps = psD.tile([128, 512], f32, tag="D", name="d_ps")
                                for _ in range(2):
                                    nc.tensor.matmul(
                                        d_ps[:],
                                        wo_sb[:, 0, 0:128],
                                        wo_sb[:, 0, :],
                                        start=True,
                                        stop=True,
                                    )
                                pt = ptp.tile([128, 1024], f32r, tag="PT", name="pt")
                                nc.scalar.activation(pt[:], s_ps[:], Exp, scale=SCALE)
                                if prev is not None:
                                    pv_pair(o_ps, prev, hp, h01, ktp - 1)
                                prev = pt
                            pv_pair(o_ps, prev, hp, h01, KTP - 1)
                            # normalize: og = O[0:64] * (1 / sums), sums bcast
                            # via K=1 matmul with a ones lhsT
                            srow = normp.tile([1, 512], f32r, tag="srow", name="srow")
                            nc.vector.tensor_copy(srow[:], o_ps[64:65, :])
                            rs_ps = psS.tile([64, 512], f32, tag="S", name="rs_ps")
                            nc.tensor.matmul(
                                rs_ps[:], ones_sb[:], srow[:], start=True, stop=True
                            )
                            rinv = normp.tile([64, 512], f32, tag="rinv", name="rinv")
                            nc.vector.reciprocal_approx_fast(rinv[:], rs_ps[:])
                            nc.vector.tensor_tensor(
                                og_sb[hp][off : off + 64, qc * 512 : (qc + 1) * 512],
                                o_ps[0:64, :],
                                rinv[:],
                                mybir.AluOpType.mult,
                            )

            # ---------------- Phase 3: output projection ----------------
            with (
                tc.tile_pool(name="ps3", bufs=4, space="PSUM") as ps3,
                tc.tile_pool(name="out3", bufs=4) as out3,
            ):
                for nt in range(NT):
                    ps = ps3.tile([128, 512], f32, tag="fin", name="fin_ps")
                    for hp in range(2):
                        nc.tensor.matmul(
                            ps[:],
                            og_sb[hp][:, nt * 128 : (nt + 1) * 128],
                            wo_sb[:, hp, :],
                            start=(hp == 0),
                            stop=(hp == 1),
                        )
                    ob = out3.tile([128, 512], f32, tag="ob", name="ob")
                    nc.vector.tensor_copy(ob[:], ps[:])
                    nc.sync.dma_start(part_d[nt * 128 : (nt + 1) * 128, :], ob[:])

    nc.compile()
    return nc


def _get_nc():
    if "nc" not in _NC_CACHE:
        _NC_CACHE["nc"] = _build()
    return _NC_CACHE["nc"]


def _prepare_in_maps(x, domain_label, W_qkv, W_d1, b_d1, W_d2, b_d2, W_out, b_out):
    x = np.asarray(x, np.float32)
    domain_label = np.asarray(domain_label, np.float32)
    W_qkv = np.asarray(W_qkv, np.float32)
    W_d1 = np.asarray(W_d1, np.float32)
    b_d1 = np.asarray(b_d1, np.float32)
    W_d2 = np.asarray(W_d2, np.float32)
    b_d2 = np.asarray(b_d2, np.float32)
    W_out = np.asarray(W_out, np.float32)

    # host: domain gate MLP + softmax over heads (tiny)
    d1 = np.maximum(domain_label @ W_d1 + b_d1, 0.0)
    d = d1 @ W_d2 + b_d2  # [B, INNER]
    d = d.reshape(B, HEADS, DH)
    e = np.exp(d - d.max(axis=1, keepdims=True))
    gate = (e / e.sum(axis=1, keepdims=True)).reshape(B, INNER).astype(np.float32)

    ones = np.ones((128, 64), np.float32)
    in_maps = []
    for c in range(NCORES):
        b, g = c // HG, c % HG
        sl = slice(g * F, (g + 1) * F)
        in_maps.append(
            {
                "xT": np.ascontiguousarray(x[b].T),
                "wq": np.ascontiguousarray(W_qkv[:, sl]),
                "wk": np.ascontiguousarray(W_qkv[:, INNER:][:, sl]),
                "wv": np.ascontiguousarray(
                    W_qkv[:, 2 * INNER :][:, sl] * gate[b, sl][None, :]
                ),
                "wo": np.ascontiguousarray(W_out[sl, :]),
                "ones": ones,
            }
        )
    return in_maps


def _run(in_maps, trace=False, tmpdir=None):
    nc = _get_nc()
    return run_bass_kernel_spmd(
        nc, in_maps, list(range(NCORES)), trace=trace, tmpdir=tmpdir
    )


def _assemble(results, b_out):
    b_out = np.asarray(b_out, np.float32)
    out = np.empty((B, N, D), np.float32)
    for b in range(B):
        out[b] = results[HG * b]["part"] + results[HG * b + 1]["part"] + b_out
    return out


def kernel(x, domain_label, W_qkv, W_d1, b_d1, W_d2, b_d2, W_out, b_out):
    in_maps = _prepare_in_maps(
        x, domain_label, W_qkv, W_d1, b_d1, W_d2, b_d2, W_out, b_out
    )
    res = _run(in_maps, trace=False)
    return _assemble(res.results, b_out)


# revision 15
# speedup vs baseline: 1.2460x; 1.0229x over previous
"""Trainium2 Bass kernel for nn_AttentionSup (dense transformer attention block).

Computation (see reference):
  qkv = x @ W_qkv; per-head attention softmax(q k^T / sqrt(d)) v;
  domain-gate (tiny MLP + softmax over heads) multiplies the attention
  output per (batch, head, dim); out = gated @ W_out + b_out.

Sharding over 8 NeuronCores: (batch b in 0..3) x (head-group g in 0..1),
4 heads per core — data-parallel over batch, tensor-parallel over heads.
Each core computes a partial output [2048, 512] for its batch from its 4
heads; the host sums the two head-group partials per batch and adds b_out
(the "all-reduce after to_out", done on host since partials per batch live
on exactly 2 cores).

The tiny domain-gate MLP ([4x4] @ [4x32] @ [32x512] per batch) is computed
on the host and folded into the V projection weights (it scales O columns,
i.e. W_v columns). Softmax uses the unnormalized-exp + ones-column trick:
V_ext = [V | 1], so PV matmul also produces row sums; normalization is a
reciprocal broadcast multiply. exp skips max-subtraction (scores ~N(0,1),
max ~5 — no overflow risk in fp32).

All matmuls run in fp32r (1 cycle/row at free-dim>=256; fp32 would be 4x
slower). fp32 PSUM accumulation throughout.
"""

import sys

sys.path.insert(0, "/opt/trn_rl_repo")

import numpy as np
from contextlib import ExitStack

import concourse.bass as bass
import concourse.tile as tile
from concourse import bacc, mybir
from concourse.bass_utils import run_bass_kernel_spmd


def _install_ntff_hook():
    """Provide antenv.axon_hooks (absent from the image) so
    run_bass_kernel_spmd(trace=True) can capture NTFF profiles under axon."""
    import types

    if "antenv.axon_hooks" in sys.modules:
        return
    mod = types.ModuleType("antenv.axon_hooks")
    mod._HOOK = None
    mod.set_axon_ntff_profile_hook = lambda h: setattr(mod, "_HOOK", h)
    mod.get_axon_ntff_profile_hook = lambda: mod._HOOK
    try:
        from trn_agent_boot.trn_boot import _ntff_profile_via_ctypes

        mod._HOOK = _ntff_profile_via_ctypes("/opt/axon/libaxon_pjrt.so")
    except Exception:
        pass
    sys.modules["antenv.axon_hooks"] = mod
    try:
        import antenv

        antenv.axon_hooks = mod
    except Exception:
        pass


_install_ntff_hook()

f32 = mybir.dt.float32
f32r = mybir.dt.float32r
Exp = mybir.ActivationFunctionType.Exp

# Problem shapes (hardcoded per contract)
B, N, D = 4, 2048, 512
HEADS, DH = 8, 64
INNER = HEADS * DH  # 512
SCALE = DH**-0.5
NCORES = 8
HG = 2  # head groups (tensor-parallel degree)
HPC = HEADS // HG  # 4 heads per core
F = HPC * DH  # 256 inner dims per core
NT = N // 128  # 16 n(token)-tiles
DT = D // 128  # 4 d-tiles
QC = 4  # q chunks of 512
KTP = NT // 2  # 8 kt-pairs

_NC_CACHE = {}


def _build():
    """Build + compile the per-core Bass program (same program on all cores)."""
    nc = bacc.Bacc("TRN2", target_bir_lowering=False, debug=False, num_devices=NCORES)

    xT_d = nc.dram_tensor("xT", [D, N], f32, kind="ExternalInput")
    wq_d = nc.dram_tensor("wq", [D, F], f32, kind="ExternalInput")
    wk_d = nc.dram_tensor("wk", [D, F], f32, kind="ExternalInput")
    wv_d = nc.dram_tensor("wv", [D, F], f32, kind="ExternalInput")  # gate-scaled
    wo_d = nc.dram_tensor("wo", [F, D], f32, kind="ExternalInput")
    ones_d = nc.dram_tensor("ones", [128, 64], f32, kind="ExternalInput")
    part_d = nc.dram_tensor("part", [N, D], f32, kind="ExternalOutput")

    with tile.TileContext(nc) as tc:
        with ExitStack() as ctx:
            persist = ctx.enter_context(tc.tile_pool(name="persist", bufs=1))

            # qT/kT: [f, n] layout, one tile per head-pair (f-tile).
            qt_sb = [
                persist.tile([128, N], f32r, tag=f"qt{i}", name=f"qt{i}")
                for i in range(2)
            ]
            kt_sb = [
                persist.tile([128, N], f32r, tag=f"kt{i}", name=f"kt{i}")
                for i in range(2)
            ]
            # V_ext natural layout: [ktok, nt, head, 64+1]
            v_sb = persist.tile([128, NT, HPC, 65], f32r, tag="v", name="v")
            # gated+normalized attention output O^T: [f, n], per head-pair
            og_sb = [
                persist.tile([128, N], f32r, tag=f"og{i}", name=f"og{i}")
                for i in range(2)
            ]
            wo_sb = persist.tile([128, 2, D], f32r, tag="wo", name="wo")
            ones_sb = persist.tile([1, 64], f32r, tag="ones1", name="ones1")

            nc.sync.dma_start(ones_sb[:], ones_d[0:1, :].bitcast(f32r))
            # Warm the ACT exp table set (~2.7us load) during the input DMAs
            # so the first real exp in phase 2 doesn't stall on it.
            warm_sb = persist.tile([1, 64], f32, tag="warm", name="warm")
            nc.scalar.activation(warm_sb[:], ones_sb[:].bitcast(f32), Exp, scale=0.0)
            nc.sync.dma_start(
                wo_sb[:], wo_d[:].rearrange("(ft p) m -> p ft m", p=128).bitcast(f32r)
            )
            # ones column of V_ext: a strided 4-byte-per-element DMA would
            # explode into 8192 descriptors and hog all DMA engines for ~40us,
            # so land ones contiguously and scatter with one DVE copy instead.
            ones64_sb = persist.tile([128, 64], f32r, tag="ones64", name="ones64")
            nc.sync.dma_start(ones64_sb[:], ones_d[:, :].bitcast(f32r))
            nc.vector.tensor_copy(
                v_sb[:].rearrange("p a b c -> p (a b) c")[:, :, 64],
                ones64_sb[:],
            )

            # ---------------- Phase 1: QKV projections ----------------
            with (
                tc.tile_pool(name="ph1", bufs=1) as ph1,
                tc.tile_pool(name="ps1", bufs=8, space="PSUM") as ps1,
            ):
                # chunk the xT DMA per d-tile AND use one tile per chunk so
                # dependency tracking lets the first matmuls start after ~1MB
                # instead of waiting for the whole 4MB
                xt_sb = [
                    ph1.tile([128, N], f32r, tag=f"xt{dt}", name=f"xt{dt}")
                    for dt in range(DT)
                ]
                w_sb = {}
                for wname, w_d in (("wq", wq_d), ("wk", wk_d), ("wv", wv_d)):
                    w_sb[wname] = [
                        ph1.tile([128, F], f32r, tag=f"{wname}{dt}", name=f"{wname}{dt}")
                        for dt in range(DT)
                    ]
                xt_r = xT_d[:].rearrange("(dt p) n -> p dt n", p=128).bitcast(f32r)
                for dt in range(DT):
                    for wname, w_d in (("wq", wq_d), ("wk", wk_d), ("wv", wv_d)):
                        nc.sync.dma_start(
                            w_sb[wname][dt][:],
                            w_d[:]
                            .rearrange("(dt p) f -> p dt f", p=128)
                            .bitcast(f32r)[:, dt],
                        )
                for dt in range(DT):
                    nc.sync.dma_start(xt_sb[dt][:], xt_r[:, dt])

                # qT/kT: [f, n] = W^T x^T ; lhsT = W[dtile, ftile], rhs =
                # xT[dtile, nchunk]. dt-outer over 8 live psum accumulators:
                # PE starts on the first xT chunk.
                for wname, dst in (("wq", qt_sb), ("wk", kt_sb)):
                    tiles = [
                        ps1.tile([128, 512], f32, tag="mm", name=f"mm_ps{i}")
                        for i in range(8)
                    ]
                    for dt in range(DT):
                        for ft in range(2):
                            for qc in range(QC):
                                nc.tensor.matmul(
                                    tiles[ft * QC + qc][:],
                                    w_sb[wname][dt][:, ft * 128 : (ft + 1) * 128],
                                    xt_sb[dt][:, qc * 512 : (qc + 1) * 512],
                                    start=(dt == 0),
                                    stop=(dt == DT - 1),
                                )
                    for ft in range(2):
                        for qc in range(QC):
                            nc.vector.tensor_copy(
                                dst[ft][:, qc * 512 : (qc + 1) * 512],
                                tiles[ft * QC + qc][:],
                            )

                # V natural: [n, f] = x W_v ; lhsT = xT[dtile, ntile], rhs = W_v[dtile, :]
                for nt in range(NT):
                    ps = ps1.tile([128, 512], f32, tag="mm", name="mmv_ps")
                    for dt in range(DT):
                        nc.tensor.matmul(
                            ps[:, 0:F],
                            xt_sb[dt][:, nt * 128 : (nt + 1) * 128],
                            w_sb["wv"][dt][:],
                            start=(dt == 0),
                            stop=(dt == DT - 1),
                        )
                    nc.vector.tensor_copy(
                        v_sb[:, nt, :, 0:64],
                        ps[:, 0:F].rearrange("p (h e) -> p h e", e=64),
                    )

            # ---------------- Phase 2: attention ----------------
            # psS bufs=3 (6 banks) lets PE run ST matmuls ~2 kt-pairs ahead
            # of the exp on ACT, keeping the tensor engine dense enough to
            # hold the HAM clock at 2.4 GHz. psO bufs=2 (2 banks) pipelines
            # the per-(head, qchunk) accumulator across iterations.
            with (
                tc.tile_pool(name="ptp", bufs=4) as ptp,
                tc.tile_pool(name="normp", bufs=2) as normp,
                tc.tile_pool(name="psS", bufs=2, space="PSUM") as psS,
                tc.tile_pool(name="psO", bufs=2, space="PSUM") as psO,
                tc.tile_pool(name="psD", bufs=1, space="PSUM") as psD,
            ):
                def pv_pair(o_ps, pt, hp, h01, ktp):
                    for j in range(2):
                        kt = 2 * ktp + j
                        nc.tensor.matmul(
                            o_ps[:],
                            v_sb[:, kt, hp * 2 + h01, :],
                            pt[:, j * 512 : (j + 1) * 512],
                            start=(kt == 0),
                            stop=(kt == NT - 1),
                        )

                for hp in range(2):
                    for qc in range(QC):
                        for h01 in range(2):
                            off = h01 * 64
                            o_ps = psO.tile([65, 512], f32, tag="O", name="o_ps")
                            # software pipeline: PV consumes the PREVIOUS
                            # kt-pair's exp output so the tensor engine never
                            # waits on the activation engine mid-stream; one
                            # keep-warm filler matmul per iteration balances
                            # PE pace to the exp pace so the HAM clock gate
                            # stays at 2.4 GHz.
                            prev = None
                            for ktp in range(KTP):
                                s_ps = psS.tile([128, 1024], f32, tag="S", name="s_ps")
                                for j in range(2):
                                    kt = 2 * ktp + j
                                    nc.tensor.matmul(
                                        s_ps[:, j * 512 : (j + 1) * 512],
                                        kt_sb[hp][
                                            off : off + 64, kt * 128 : (kt + 1) * 128
                                        ],
                                        qt_sb[hp][
                                            off : off + 64, qc * 512 : (qc + 1) * 512
                                        ],
                                        start=True,
                                        stop=True,
                                    )
                                # two keep-warm filler matmuls per iteration:
                                # attention is exp(ACT)-paced; without dense
                                # PE work the HAM clock gate falls to 1.2 GHz
                                # (2x matmul slowdown). Two junk matmuls keep
                                # the PE strictly busier than ACT so the
                                # activity window stays warm.
                                d_ps = psD.tile([128, 512], f32, tag="D", name="d_ps")
                                for _ in range(2):
                                    nc.tensor.matmul(
                                        d_ps[:],
                                        wo_sb[:, 0, 0:128],
                                        wo_sb[:, 0, :],
                                        start=True,
                                        stop=True,
                                    )
                                pt = ptp.tile([128, 1024], f32r, tag="PT", name="pt")
                                nc.scalar.activation(pt[:], s_ps[:], Exp, scale=SCALE)
                                if prev is not None:
                                    pv_pair(o_ps, prev, hp, h01, ktp - 1)
                                prev = pt
                            pv_pair(o_ps, prev, hp, h01, KTP - 1)
                            # normalize: og = O[0:64] * (1 / sums), sums bcast
                            # via K=1 matmul with a ones lhsT
                            srow = normp.tile([1, 512], f32r, tag="srow", name="srow")
                            nc.vector.tensor_copy(srow[:], o_ps[64:65, :])
                            rs_ps = psS.tile([64, 512], f32, tag="S", name="rs_ps")
                            nc.tensor.matmul(
                                rs_ps[:], ones_sb[:], srow[:], start=True, stop=True
                            )
                            rinv = normp.tile([64, 512], f32, tag="rinv", name="rinv")
                            nc.vector.reciprocal_approx_fast(rinv[:], rs_ps[:])
                            nc.vector.tensor_tensor(
                                og_sb[hp][off : off + 64, qc * 512 : (qc + 1) * 512],
                                o_ps[0:64, :],
                                rinv[:],
                                mybir.AluOpType.mult,
                            )

            # ---------------- Phase 3: output projection ----------------
            with (
                tc.tile_pool(name="ps3", bufs=4, space="PSUM") as ps3,
                tc.tile_pool(name="out3", bufs=4) as out3,
            ):
                for nt in range(NT):
                    ps = ps3.tile([128, 512], f32, tag="fin", name="fin_ps")
                    for hp in range(2):
                        nc.tensor.matmul(
                            ps[:],
                            og_sb[hp][:, nt * 128 : (nt + 1) * 128],
                            wo_sb[:, hp, :],
                            start=(hp == 0),
                            stop=(hp == 1),
                        )
                    ob = out3.tile([128, 512], f32, tag="ob", name="ob")
                    nc.vector.tensor_copy(ob[:], ps[:])
                    nc.sync.dma_start(part_d[nt * 128 : (nt + 1) * 128, :], ob[:])

    nc.compile()
    return nc


def _get_nc():
    if "nc" not in _NC_CACHE:
        _NC_CACHE["nc"] = _build()
    return _NC_CACHE["nc"]


def _prepare_in_maps(x, domain_label, W_qkv, W_d1, b_d1, W_d2, b_d2, W_out, b_out):
    x = np.asarray(x, np.float32)
    domain_label = np.asarray(domain_label, np.float32)
    W_qkv = np.asarray(W_qkv, np.float32)
    W_d1 = np.asarray(W_d1, np.float32)
    b_d1 = np.asarray(b_d1, np.float32)
    W_d2 = np.asarray(W_d2, np.float32)
    b_d2 = np.asarray(b_d2, np.float32)
    W_out = np.asarray(W_out, np.float32)

    # host: domain gate MLP + softmax over heads (tiny)
    d1 = np.maximum(domain_label @ W_d1 + b_d1, 0.0)
    d = d1 @ W_d2 + b_d2  # [B, INNER]
    d = d.reshape(B, HEADS, DH)
    e = np.exp(d - d.max(axis=1, keepdims=True))
    gate = (e / e.sum(axis=1, keepdims=True)).reshape(B, INNER).astype(np.float32)

    ones = np.ones((128, 64), np.float32)
    in_maps = []
    for c in range(NCORES):
        b, g = c // HG, c % HG
        sl = slice(g * F, (g + 1) * F)
        in_maps.append(
            {
                "xT": np.ascontiguousarray(x[b].T),
                "wq": np.ascontiguousarray(W_qkv[:, sl]),
                "wk": np.ascontiguousarray(W_qkv[:, INNER:][:, sl]),
                "wv": np.ascontiguousarray(
                    W_qkv[:, 2 * INNER :][:, sl] * gate[b, sl][None, :]
                ),
                "wo": np.ascontiguousarray(W_out[sl, :]),
                "ones": ones,
            }
        )
    return in_maps


def _run(in_maps, trace=False, tmpdir=None):
    nc = _get_nc()
    return run_bass_kernel_spmd(
        nc, in_maps, list(range(NCORES)), trace=trace, tmpdir=tmpdir
    )


def _assemble(results, b_out):
    b_out = np.asarray(b_out, np.float32)
    out = np.empty((B, N, D), np.float32)
    for b in range(B):
        out[b] = results[HG * b]["part"] + results[HG * b + 1]["part"] + b_out
    return out


def kernel(x, domain_label, W_qkv, W_d1, b_d1, W_d2, b_d2, W_out, b_out):
    in_maps = _prepare_in_maps(
        x, domain_label, W_qkv, W_d1, b_d1, W_d2, b_d2, W_out, b_out
    )
    res = _run(in_maps, trace=False)
    return _assemble(res.results, b_out)
